# revision 4
# baseline (speedup 1.0000x reference)
"""Multi-head self-attention (B=2, S=2048, D=1024, H=16, causal) on 8 NeuronCores.

Sharding: core c = 4*b + g handles batch b and heads 4g..4g+3 (batch x
head-group parallel).  Per core:
  - q/k projections in transposed layout qT/kT [dh, s], v projection in
    natural layout [s, dh] with a fused ones-column per head (softmax
    denominator falls out of the AV matmul for free)
  - causal attention in scoresT [j, i] orientation: PE scores -> ACT exp
    (scale=1/8, no max subtraction; bf16 probs have fp32 range) -> DVE
    causal-mask multiplies on diagonal blocks -> PE AV accumulation, with
    128-column trimming of the strictly-upper triangle
  - normalization by the per-query denominator via gpsimd partition
    broadcast + DVE multiply, one group behind the PE stream
  - partial o-projection out_c = merged_c @ Wo[:, cols_c].T, DMA'd bf16;
    host sums the 4 partials per batch (the only cross-core reduction)

Schedule: attention is ACT-bound (the exp stream is ~87us) while the
projections are PE-bound, so projection / o-projection PSUM groups are
WOVEN between individual attention score/AV pairs on a per-pair credit --
both engine streams stay saturated instead of stalling at coarse phase
boundaries.  Attention groups run in ascending-ic order so the exp stream
starts as soon as the first x quarter lands.  Input DMA is trickled
(quarters 2/3 of x and Wo load mid-kernel) and output DMA is spread via
op-step priority: concentrated DMA bursts alongside dense compute trip the
HAM activity monitor, which otherwise clamps the PE to half rate for tens
of microseconds.  The tail closes with a kc-split o-projection so the
final normalize chain overlaps the last matmul work.

All matmul operands are bf16 (fp32 accumulation in PSUM); fp8/DoubleRow
was measured 2x on PE but fails the 2e-2 tolerance (any single stage in
e4m3 alone costs ~2.5e-2).
"""

import ml_dtypes
import numpy as np

import concourse.bass as bass
from concourse import bacc
import concourse.mybir as mybir
import concourse.tile as tile
from concourse import bass_utils

F32 = mybir.dt.float32
BF16 = mybir.dt.bfloat16
NP_BF16 = ml_dtypes.bfloat16
EXP = mybir.ActivationFunctionType.Exp

B, S, D = 2, 2048, 1024
H, DH = 16, 64
NCORES = 8
HPG = 4                  # heads per group (per core)
M = HPG * DH             # 256 per-core head dims
DC = D // 128            # 8 contraction chunks for projections
IC = 512                 # i (query) chunk for attention
JC = 128                 # j (key) chunk for attention
SCALE = 1.0 / np.sqrt(DH)

# weave pacing estimates (ns)
PAIR_PE = 900
PAIR_ACT = 1100
STEP_PE = {"qk": 1800, "v": 950, "op": 520}


def _build_nc():
    nc = bacc.Bacc("TRN2", target_bir_lowering=False, debug=False)

    xT_d = nc.dram_tensor("xT", [D, S], BF16, kind="ExternalInput").ap()
    wqkv_d = nc.dram_tensor("wqkvT", [D, 3 * M], BF16, kind="ExternalInput").ap()
    woT_d = nc.dram_tensor("woT", [M, D], BF16, kind="ExternalInput").ap()
    mask_d = nc.dram_tensor("mask", [JC, 1536], BF16, kind="ExternalInput").ap()
    onesa_d = nc.dram_tensor("ones_a", [1, 64], BF16, kind="ExternalInput").ap()
    onesb_d = nc.dram_tensor("ones_b", [JC, HPG], BF16, kind="ExternalInput").ap()
    out_d = nc.dram_tensor("out", [S, D], BF16, kind="ExternalOutput").ap()

    with tile.TileContext(nc) as tc:
        _body(tc, xT_d, wqkv_d, woT_d, mask_d, onesa_d, onesb_d, out_d)
    nc.compile()
    return nc


def _body(tc, xT_d, wqkv_d, woT_d, mask_d, onesa_d, onesb_d, out_d):
    nc = tc.nc
    from contextlib import ExitStack
    ctx = ExitStack()
    with ctx:
        p_x = ctx.enter_context(tc.tile_pool(name="x", bufs=DC))
        p_w = ctx.enter_context(tc.tile_pool(name="w", bufs=DC))
        p_wo = ctx.enter_context(tc.tile_pool(name="wo", bufs=2))
        p_qk = ctx.enter_context(tc.tile_pool(name="qk", bufs=2))
        p_v = ctx.enter_context(tc.tile_pool(name="v", bufs=S // JC))
        p_mg = ctx.enter_context(tc.tile_pool(name="mg", bufs=2))
        p_probs = ctx.enter_context(tc.tile_pool(name="probs", bufs=6))
        p_small = ctx.enter_context(tc.tile_pool(name="small", bufs=1))
        p_mask = ctx.enter_context(tc.tile_pool(name="mask", bufs=1))
        p_ostg = ctx.enter_context(tc.tile_pool(name="ostg", bufs=2))
        p_o32 = ctx.enter_context(tc.tile_pool(name="o32", bufs=4))
        p_ones = ctx.enter_context(tc.tile_pool(name="ones", bufs=1))

        ps_big = ctx.enter_context(tc.tile_pool(name="psb", bufs=2, space="PSUM"))
        ps_sc = ctx.enter_context(tc.tile_pool(name="pss", bufs=2, space="PSUM"))
        ps_at = ctx.enter_context(tc.tile_pool(name="psa", bufs=2, space="PSUM"))

        # ---- HAM pre-warm (see v1): keep the PE activity monitor busy while
        # the first input tiles land so the clock gate is at full rate.
        wrm = p_ones.tile([128, 512], BF16, tag="warm")
        nc.vector.memset(wrm[:], 1.0)
        wrm_ps = ps_at.tile([128, 512], F32, tag="attn", name="warmps")
        NWARM = 6
        for r in range(NWARM):
            nc.tensor.matmul(wrm_ps[:], wrm[:, 0:128], wrm[:],
                             start=(r == 0), stop=(r == NWARM - 1))
        nc.scalar.copy(wrm[:, 0:1], wrm_ps[:, 0:1])  # keep alive vs DCE

        # ---- input loads: (w, x-quarter-0) interleaved per d-chunk, then
        # quarter 1.  Quarters 2 and 3 + wo are TRICKLED later (emitted
        # between attention groups) -- ascending ic order doesn't need them
        # until ~50% into the kernel, and spreading the DMA activity out
        # keeps the HAM power monitor from clamping the PE to half rate
        # during the landing window.
        w_t, x_t = [], []
        for dc in range(DC):
            w_t.append(p_w.tile([128, 3 * M], BF16, tag="w", name=f"w{dc}"))
            x_t.append(p_x.tile([128, S], BF16, tag="x", name=f"x{dc}"))
        for dc in range(4):
            nc.sync.dma_start(w_t[dc][:, 0:2 * M],
                              wqkv_d[dc * 128:(dc + 1) * 128, 0:2 * M])
            nc.sync.dma_start(x_t[dc][:, 0:512],
                              xT_d[dc * 128:(dc + 1) * 128, 0:512])
        for dc in range(4, DC):
            nc.sync.dma_start(w_t[dc][:], wqkv_d[dc * 128:(dc + 1) * 128, :])
            nc.sync.dma_start(x_t[dc][:, 0:512],
                              xT_d[dc * 128:(dc + 1) * 128, 0:512])
        for dc in range(4):
            nc.sync.dma_start(w_t[dc][:, 2 * M:3 * M],
                              wqkv_d[dc * 128:(dc + 1) * 128, 2 * M:3 * M])
        mask_t = p_mask.tile([JC, 1536], BF16, tag="mask")
        nc.sync.dma_start(mask_t[:], mask_d[:])
        ones_t = p_ones.tile([1, 64], BF16, tag="ones")
        nc.sync.dma_start(ones_t[:], onesa_d[:])
        onesb_t = p_ones.tile([JC, HPG], BF16, tag="onesb")
        nc.sync.dma_start(onesb_t[:], onesb_d[:])
        wo_t = [p_wo.tile([128, D], BF16, tag="wo", name=f"wo{kc}")
                for kc in range(2)]

        def late_dma(which):
            # deferred input loads, emitted between attention groups
            if which == "xq1":
                for dc in range(DC):
                    nc.sync.dma_start(x_t[dc][:, 512:1024],
                                      xT_d[dc * 128:(dc + 1) * 128, 512:1024])
            elif which == "xq2":
                for dc in range(DC):
                    nc.sync.dma_start(x_t[dc][:, 1024:1536],
                                      xT_d[dc * 128:(dc + 1) * 128, 1024:1536])
            elif which == "xq3":
                for dc in range(DC):
                    nc.sync.dma_start(x_t[dc][:, 1536:2048],
                                      xT_d[dc * 128:(dc + 1) * 128, 1536:2048])
            elif which == "wo":
                for kc in range(2):
                    nc.sync.dma_start(wo_t[kc][:],
                                      woT_d[kc * 128:(kc + 1) * 128, :])
            else:
                raise AssertionError(f"unknown late_dma stage {which}")

        # ---- building blocks ----
        q_t = {mc: p_qk.tile([128, S], BF16, tag="qT", name=f"qT{mc}")
               for mc in range(2)}
        k_t = {mc: p_qk.tile([128, S], BF16, tag="kT", name=f"kT{mc}")
               for mc in range(2)}
        evict_flip = [0]

        def evict(dst, src, dve_only=False):
            # PSUM evictions: alternate DVE / ACT only while ACT is idle
            # (era-A, before the exp stream saturates it); DVE-only later.
            if dve_only:
                nc.vector.tensor_copy(dst, src)
                return
            evict_flip[0] ^= 1
            if evict_flip[0]:
                nc.vector.tensor_copy(dst, src)
            else:
                nc.scalar.copy(dst, src)

        def qk_step(which, mc, s4, split=False):
            # one 512-col block of qT/kT m-chunk mc: 8 (or 2x4) contraction
            # matmuls + eviction.  split: two half-contractions merged at
            # eviction so the first blocks can start before all x d-chunks
            # land (era-A DMA overlap).
            woff = 0 if which == "q" else M
            dst = q_t[mc] if which == "q" else k_t[mc]
            sl = slice(s4 * 512, (s4 + 1) * 512)
            psa = ps_big.tile([128, 512], F32, tag="proj")
            nd = DC // 2 if split else DC
            for dc in range(nd):
                nc.tensor.matmul(
                    psa[:],
                    w_t[dc][:, woff + mc * 128:woff + (mc + 1) * 128],
                    x_t[dc][:, sl],
                    start=(dc == 0), stop=(dc == nd - 1))
            if split:
                psb = ps_sc.tile([128, 2 * IC], F32, tag="scores")
                for dc in range(DC // 2, DC):
                    nc.tensor.matmul(
                        psb[:, 0:512],
                        w_t[dc][:, woff + mc * 128:woff + (mc + 1) * 128],
                        x_t[dc][:, sl],
                        start=(dc == DC // 2), stop=(dc == DC - 1))
                nc.scalar.copy(dst[:, sl], psa[:])
                nc.vector.tensor_add(dst[:, sl], dst[:, sl], psb[:, 0:512])
            else:
                evict(dst[:, sl], psa[:], dve_only=(s4 >= 2))

        v_t = {}

        def v_step(sc):
            # v[s, m] tile for j-chunk sc with the per-head ones column
            vt = p_v.tile([JC, HPG * (DH + 1)], BF16, tag="v", name=f"v{sc}")
            nc.vector.tensor_copy(
                vt[:].rearrange("p (h e) -> p h e", h=HPG)[:, :, DH:DH + 1].squeeze(2),
                onesb_t[:])
            psa = ps_big.tile([128, 512], F32, tag="proj")
            for dc in range(DC):
                nc.tensor.matmul(
                    psa[:, 0:M],
                    x_t[dc][:, sc * 128:(sc + 1) * 128],
                    w_t[dc][:, 2 * M:3 * M],
                    start=(dc == 0), stop=(dc == DC - 1))
            src = psa[:, 0:M].rearrange("p (h d) -> p h d", h=HPG)
            dst = vt[:].rearrange("p (h e) -> p h e", h=HPG)[:, :, 0:DH]
            evict(dst, src, dve_only=(sc >= 8))
            v_t[sc] = vt

        mg_t = [p_mg.tile([128, S], BF16, tag="mgT", name=f"mg{i}")
                for i in range(M // 128)]

        def score_pair(h, ic, ja, jb):
            qk_tile = h // 2
            prow = 64 * (h % 2)
            sc_ps = ps_sc.tile([128, 2 * IC], F32, tag="scores")
            pr = p_probs.tile([JC, 2 * IC], BF16, tag="probs")
            deltas = []
            for u, jc in enumerate(range(ja, jb)):
                dlt = max(0, jc * JC - ic * IC)
                deltas.append(dlt)
                nc.tensor.matmul(
                    sc_ps[:, u * IC + dlt:(u + 1) * IC],
                    k_t[qk_tile][prow:prow + DH, jc * JC:(jc + 1) * JC],
                    q_t[qk_tile][prow:prow + DH,
                                 ic * IC + dlt:(ic + 1) * IC],
                    start=True, stop=True)
            if deltas[-1] == 0:
                nc.scalar.activation(pr[:], sc_ps[:], EXP, scale=SCALE)
            else:
                for u, dlt in enumerate(deltas):
                    nc.scalar.activation(
                        pr[:, u * IC + dlt:(u + 1) * IC],
                        sc_ps[:, u * IC + dlt:(u + 1) * IC],
                        EXP, scale=SCALE)
            delta0 = ja * JC - ic * IC
            if delta0 == 0:        # diagonal pair A: deltas 0 and 128
                pv = pr[:].rearrange("p (a b) -> p a b", a=2)[:, :, 0:256]
                nc.vector.tensor_mul(
                    pv, pv,
                    mask_t[:, 0:512].rearrange("p (a b) -> p a b", a=2))
            elif delta0 == 256:    # diagonal pair B: deltas 256 and 384
                nc.vector.tensor_mul(
                    pr[:], pr[:], mask_t[:, 512:1536])
            return pr, deltas

        def av_pair(h, ic, at_ps, ja, jb, pr, deltas, njc):
            for u, jc in enumerate(range(ja, jb)):
                dlt = deltas[u]
                nc.tensor.matmul(
                    at_ps[:, dlt:IC],
                    v_t[jc][:, h * (DH + 1):(h + 1) * (DH + 1)],
                    pr[:, u * IC + dlt:(u + 1) * IC],
                    start=(jc == 0), stop=(jc == njc - 1),
                    skip_group_check=True)

        def normalize(h, ic, at_ps, use_pe=False):
            qk_tile = h // 2
            prow = 64 * (h % 2)
            den = p_small.tile([1, IC], F32, tag="den")
            nc.vector.tensor_copy(den[:], at_ps[DH:DH + 1, :])
            rc32 = p_small.tile([1, IC], F32, tag="recip32")
            nc.vector.reciprocal_approx_fast(rc32[:], den[:])
            bc_sb = p_small.tile([DH, IC], F32, tag="bcast")
            if use_pe:
                rc = p_small.tile([1, IC], BF16, tag="recip")
                nc.vector.tensor_copy(rc[:], rc32[:])
                bc_ps = ps_big.tile([DH, IC], F32, tag="proj")
                nc.tensor.matmul(bc_ps[:], ones_t[:], rc[:],
                                 start=True, stop=True)
                nc.vector.tensor_copy(bc_sb[:], bc_ps[:])
            else:
                nc.gpsimd.partition_broadcast(bc_sb[:], rc32[:])
            nc.vector.tensor_mul(
                mg_t[qk_tile][prow:prow + DH, ic * IC:(ic + 1) * IC],
                at_ps[0:DH, :], bc_sb[:])

        def oproj_half(sc, nn):
            # one [128, 512] half of the o-projection for s-chunk sc
            stg = p_ostg.tile([128, D], BF16, tag="ostg", name=f"ostg{sc}")
            ps = ps_big.tile([128, 512], F32, tag="proj")
            for kc in range(2):
                nc.tensor.matmul(
                    ps[:],
                    mg_t[kc][:, sc * 128:(sc + 1) * 128],
                    wo_t[kc][:, nn * 512:(nn + 1) * 512],
                    start=(kc == 0), stop=(kc == 1))
            sl = slice(nn * 512, (nn + 1) * 512)
            evict(stg[:, sl], ps[:], dve_only=True)
            nc.sync.dma_start(out_d[sc * 128:(sc + 1) * 128, sl], stg[:, sl])

        def oproj0_kc0(sc):
            # first half of the closing o-proj contraction (heads 0,1 only)
            stg32 = p_o32.tile([128, D], F32, tag="stg32", name=f"o32_{sc}")
            for nn in range(2):
                ps = ps_big.tile([128, 512], F32, tag="proj")
                nc.tensor.matmul(
                    ps[:], mg_t[0][:, sc * 128:(sc + 1) * 128],
                    wo_t[0][:, nn * 512:(nn + 1) * 512], start=True, stop=True)
                nc.vector.tensor_copy(stg32[:, nn * 512:(nn + 1) * 512], ps[:])
            return stg32

        def oproj0_kc1(sc, stg32):
            # second half (heads 2,3) + merge + per-half DMA
            stg = p_ostg.tile([128, D], BF16, tag="ostg")
            for nn in range(2):
                ps = ps_big.tile([128, 512], F32, tag="proj")
                nc.tensor.matmul(
                    ps[:], mg_t[1][:, sc * 128:(sc + 1) * 128],
                    wo_t[1][:, nn * 512:(nn + 1) * 512], start=True, stop=True)
                sl = slice(nn * 512, (nn + 1) * 512)
                nc.vector.tensor_add(stg[:, sl], stg32[:, sl], ps[:])
                nc.sync.dma_start(out_d[sc * 128:(sc + 1) * 128, sl],
                                  stg[:, sl])

        # ---- woven schedule ----
        # Projection steps and o-projection halves are emitted between
        # attention pairs on a per-pair PE credit, so the exp stream (ACT)
        # and the matmul stream (PE) stay simultaneously busy and the
        # output DMA is spread across the whole run instead of piling up
        # at the end (which trips the HAM activity clamp).
        proj_q = []          # static projection steps, in consumption order
        proj_q += [("qk", ("q", 0, 0, True)), ("qk", ("k", 0, 0, True))]
        proj_q += [("v", (sc,)) for sc in range(0, 4)]
        proj_q += [("qk", ("q", 1, 0, False)), ("qk", ("k", 1, 0, False))]
        proj_q += [("qk", (w, mc, 1, False)) for w in ("q", "k") for mc in (0, 1)]
        proj_q += [("v", (sc,)) for sc in range(4, 8)]
        proj_q += [("qk", (w, mc, 2, False)) for w in ("q", "k") for mc in (0, 1)]
        proj_q += [("v", (sc,)) for sc in range(8, 12)]
        proj_q += [("qk", (w, mc, 3, False)) for w in ("q", "k") for mc in (0, 1)]
        proj_q += [("v", (sc,)) for sc in range(12, 16)]
        emitted = [False] * len(proj_q)
        op_q = []            # dynamic o-proj steps (sc, nn), FIFO
        dma_epoch = [0]      # x quarters 0..dma_epoch are DMA-emitted

        def step_epoch(item):
            kind, args = item
            if kind == "qk":
                return args[2]
            return args[0] // 4

        def run_step(item):
            kind, args = item
            if kind == "qk":
                qk_step(*args)
            elif kind == "v":
                v_step(*args)

        def emit_proj(i):
            if not emitted[i]:
                emitted[i] = True
                run_step(proj_q[i])
                return STEP_PE[proj_q[i][0]]
            return 0

        def prereq(h, ic):
            mc = h // 2
            for i, (kind, args) in enumerate(proj_q):
                if kind == "qk" and args[1] == mc and args[2] <= ic:
                    emit_proj(i)
                if kind == "v" and args[0] < 4 * (ic + 1):
                    emit_proj(i)

        credit = [0.0]

        def weave():
            # op steps first (spreads the output DMA), then projections
            while credit[0] > 0 and op_q:
                oproj_half(*op_q.pop(0))
                credit[0] -= STEP_PE["op"]
            for i in range(len(proj_q)):
                if credit[0] <= 0:
                    return
                if not emitted[i] and step_epoch(proj_q[i]) <= dma_epoch[0]:
                    credit[0] -= emit_proj(i)

        groups = [(h, ic) for ic in range(4) for h in range(HPG)]
        pending = None           # (h, ic, at_ps) awaiting normalize
        stg32s = {}
        stg32s = {}
        for gi, (h, ic) in enumerate(groups):
            # trickled input DMA, one phase ahead of consumption
            if (h, ic) == (1, 0):
                late_dma("xq1")
                dma_epoch[0] = 1
            elif (h, ic) == (3, 0):
                late_dma("wo")
            elif (h, ic) == (0, 1):
                late_dma("xq2")
                dma_epoch[0] = 2
            elif (h, ic) == (0, 2):
                late_dma("xq3")
                dma_epoch[0] = 3
            njc = 4 * (ic + 1)
            at_ps = ps_at.tile([DH + 1, IC], F32, tag="attn")
            pairs = [(p, min(p + 2, njc)) for p in range(0, njc, 2)]
            prev_pr = None
            for pi, (ja, jb) in enumerate(pairs):
                prereq(h, ic)
                pr, deltas = score_pair(h, ic, ja, jb)
                credit[0] += 750 if ic < 3 else 900
                if prev_pr is not None:
                    (pja, pjb), ppr, pdl = prev_pr
                    av_pair(h, ic, at_ps, pja, pjb, ppr, pdl, njc)
                prev_pr = ((ja, jb), pr, deltas)
                if pi == 0 and pending is not None:
                    normalize(*pending)
                    ph, pic = pending[0], pending[1]
                    if (ph, pic) == (1, 3):
                        # heads 0,1 of ic3 normalized: their half of the
                        # closing o-projection can weave into the remaining
                        # ic3 groups (one block per pair below)
                        for sc in range(12, 16):
                            stg32s[sc] = None
                    if ph == HPG - 1:
                        # all heads of pic normalized: o-proj cols ready
                        for sc in range(4 * pic, 4 * pic + 4):
                            for nn in range(2):
                                op_q.append((sc, nn))
                    pending = None
                if stg32s and None in stg32s.values() and pi >= 1:
                    for sc in sorted(stg32s):
                        if stg32s[sc] is None:
                            stg32s[sc] = oproj0_kc0(sc)
                            break
                weave()
            # flush the last AV of the group
            (ja, jb), pr, deltas = prev_pr
            av_pair(h, ic, at_ps, ja, jb, pr, deltas, njc)
            pending = (h, ic, at_ps)

        # ---- endgame: flush leftovers, normalize (h3, ic3) via the
        # PE-broadcast drain path, close with the kc-split o-projection.
        for i in range(len(proj_q)):
            emit_proj(i)
        while op_q:
            oproj_half(*op_q.pop(0))
        for sc in range(12, 16):
            if stg32s.get(sc) is None:
                stg32s[sc] = oproj0_kc0(sc)
        h, ic, at_ps = pending
        normalize(h, ic, at_ps, use_pe=True)
        for sc in range(12, 16):
            oproj0_kc1(sc, stg32s[sc])


_NC_CACHE = None


def _get_nc():
    global _NC_CACHE
    if _NC_CACHE is None:
        _NC_CACHE = _build_nc()
    return _NC_CACHE


def _causal_mask_tile():
    j = np.arange(JC)[:, None]
    c = np.arange(896)[None, :]
    big = (j <= c - 384).astype(np.float32)
    return np.concatenate(
        [big[:, 384:640], big[:, 256:512], big[:, 128:640], big[:, 0:512]],
        axis=1)


def _prepare_in_maps(inputs):
    x = np.asarray(inputs["in_features"], dtype=np.float32)
    wqT = np.ascontiguousarray(np.asarray(inputs["q_proj_weight"], np.float32).T)
    wkT = np.ascontiguousarray(np.asarray(inputs["k_proj_weight"], np.float32).T)
    wvT = np.ascontiguousarray(np.asarray(inputs["v_proj_weight"], np.float32).T)
    woT = np.ascontiguousarray(np.asarray(inputs["o_proj_weight"], np.float32).T)
    xT = [np.ascontiguousarray(x[b].T).astype(NP_BF16) for b in range(B)]
    mask = _causal_mask_tile().astype(NP_BF16)

    in_maps = []
    for c in range(NCORES):
        b, g = divmod(c, HPG)
        ms = slice(g * M, (g + 1) * M)
        in_maps.append({
            "xT": xT[b],
            "wqkvT": np.ascontiguousarray(
                np.concatenate([wqT[:, ms], wkT[:, ms], wvT[:, ms]],
                               axis=1)).astype(NP_BF16),
            "woT": np.ascontiguousarray(woT[ms, :]).astype(NP_BF16),
            "mask": mask,
            "ones_a": np.ones((1, 64), NP_BF16),
            "ones_b": np.ones((JC, HPG), NP_BF16),
        })
    return in_maps


def kernel(q_proj_weight, k_proj_weight, v_proj_weight, o_proj_weight, in_features):
    in_dtype = np.asarray(in_features).dtype
    in_maps = _prepare_in_maps({
        "q_proj_weight": q_proj_weight,
        "k_proj_weight": k_proj_weight,
        "v_proj_weight": v_proj_weight,
        "o_proj_weight": o_proj_weight,
        "in_features": in_features,
    })
    nc = _get_nc()
    res = bass_utils.run_bass_kernel_spmd(nc, in_maps, core_ids=list(range(NCORES)))
    out = np.zeros((B, S, D), dtype=np.float32)
    for c in range(NCORES):
        out[c // HPG] += res.results[c]["out"]
    return out.astype(in_dtype)
